# revision 20
# baseline (speedup 1.0000x reference)
"""2-layer GAT on 8 Trainium2 NeuronCores (Bass/Tile).

Sharding: edges sorted by destination node; nodes partitioned 8 x N/8 across
cores (dst-partitioned edge-parallel). Per-dst softmax groups stay entirely on
one core, so aggregation needs no cross-core reduction. Per 128-node block:
dma_gather of per-edge source feature rows from a replicated table (two halves
so indices fit int16), a one-hot S_T built in one batched DVE op per block,
and the weighted scatter-add + softmax denominator computed as a single
PSUM-accumulated bf16 matmul per 128-edge tile (the per-edge weight is stashed
in a spare column of the gathered rows so numerator and denominator share the
matmul). One AllGather shares the small layer-2 feature table between layers.
"""
import numpy as np

P = 128
NCORES = 8
GRP = 2
GCH = 8  # max gather columns (x128 idxs) per dma_gather call

_CACHE = {}


def _wrap_idx_segments(segs, total_cols):
    arr = np.zeros((16, total_cols), np.int16)
    for off, idx in segs:
        n = len(idx)
        if n:
            arr[:, off:off + n // 16] = idx.reshape(n // 16, 16).T
    return np.tile(arr, (8, 1))


def _prep(x, edge_index):
    N = x.shape[0]
    NPC = N // NCORES
    NB = (NPC + P - 1) // P
    SPLIT = N // 2

    src = np.concatenate([np.asarray(edge_index[0]), np.arange(N, dtype=np.int64)])
    dst = np.concatenate([np.asarray(edge_index[1]), np.arange(N, dtype=np.int64)])
    order = np.argsort(dst, kind="stable")
    s_all = src[order].astype(np.int64)
    d_all = dst[order].astype(np.int64)

    lists = [[[None, None] for _ in range(NB)] for _ in range(NCORES)]
    for c in range(NCORES):
        base = c * NPC
        for b in range(NB):
            e0 = np.searchsorted(d_all, base + b * P)
            e1 = np.searchsorted(d_all, min(base + (b + 1) * P, base + NPC))
            ss, dd = s_all[e0:e1], d_all[e0:e1]
            m = ss < SPLIT
            for s, sel in ((0, m), (1, ~m)):
                sv, dv = ss[sel], dd[sel]
                o = np.argsort(sv, kind="stable")  # ascending src: HBM locality
                lists[c][b][s] = (sv[o], dv[o])

    NT = np.zeros((NB, 2), np.int64)
    for b in range(NB):
        for s in range(2):
            mx = max(len(lists[c][b][s][0]) for c in range(NCORES))
            NT[b, s] = (mx + P - 1) // P

    groups = [list(range(g, min(g + GRP, NB))) for g in range(0, NB, GRP)]

    tile_of = np.zeros((NB, 2), np.int64)
    t = 0
    for b in range(NB):
        for s in range(2):
            tile_of[b, s] = t
            t += int(NT[b, s])
    NTOT = t

    g_cols, g_off = 0, []
    for g, blocks in enumerate(groups):
        offs = []
        for s in range(2):
            ntg = int(sum(NT[b, s] for b in blocks))
            offs.append((g_cols, ntg))
            g_cols += ntg * 8
        g_off.append(offs)
    d_cols, d_off = 0, []
    for g, blocks in enumerate(groups):
        ntg = int(sum(NT[b, 0] + NT[b, 1] for b in blocks))
        d_off.append((d_cols, ntg))
        d_cols += ntg * 8

    plan = dict(N=N, NPC=NPC, NB=NB, SPLIT=SPLIT, NT=NT, groups=groups,
                tile_of=tile_of, NTOT=NTOT, g_off=g_off, d_off=d_off,
                g_cols=g_cols, d_cols=d_cols)

    per_core = []
    for c in range(NCORES):
        base = c * NPC
        gsegs, dsegs = [], []
        d_fp = np.full((NTOT, P), -1.0, np.float32)
        for g, blocks in enumerate(groups):
            for s in range(2):
                col0, ntg = g_off[g][s]
                idx = np.zeros(ntg * P, np.int64)
                pos = 0
                for b in blocks:
                    ss = lists[c][b][s][0]
                    nslots = int(NT[b, s]) * P
                    idx[pos:pos + len(ss)] = ss - (SPLIT if s == 1 else 0)
                    pos += nslots
                gsegs.append((col0, idx.astype(np.int16)))
            col0, ntg = d_off[g]
            didx = np.zeros(ntg * P, np.int64)
            pos = 0
            for b in blocks:
                for s in range(2):
                    ss, dd = lists[c][b][s]
                    nslots = int(NT[b, s]) * P
                    didx[pos:pos + len(dd)] = dd - base
                    pos += nslots
                    t0 = int(tile_of[b, s])
                    dv = np.full(nslots, -1.0, np.float32)
                    dv[:len(dd)] = (dd - base - b * P).astype(np.float32)
                    d_fp[t0:t0 + int(NT[b, s])] = dv.reshape(int(NT[b, s]), P)
            dsegs.append((col0, didx.astype(np.int16)))
        per_core.append(dict(
            g_idx=_wrap_idx_segments(gsegs, g_cols),
            dl_idx=_wrap_idx_segments(dsegs, d_cols),
            d_fp=d_fp,
        ))
    return plan, per_core


def _build(plan, dims, has_b1, has_b2):
    import concourse.bass as bass
    import concourse.bacc as bacc
    import concourse.tile as tile
    from concourse import mybir

    f32 = mybir.dt.float32
    bf16 = mybir.dt.bfloat16
    i16 = mybir.dt.int16
    AF = mybir.ActivationFunctionType
    OP = mybir.AluOpType

    N, NPC, NB, SPLIT = plan["N"], plan["NPC"], plan["NB"], plan["SPLIT"]
    NT, groups, tile_of, NTOT = (plan["NT"], plan["groups"], plan["tile_of"],
                                 plan["NTOT"])
    HID, H1, C1, OUT = dims["HID"], dims["H1"], dims["C1"], dims["OUT"]
    NPAD = ((N + P - 1) // P) * P
    NBA = NPAD // P
    ROW1 = 384
    W1C = HID + H1          # h | src-score-hi  (lo computed on device)
    DEN1 = HID + 2 * H1     # stash column for the fused denominator
    ROW2 = 128
    W2C = OUT + 2           # h2 | s2src | s2dst  (f32 epilogue values)
    DEN2 = OUT + 4          # h2row: [h2, shi, dhi, slo, dlo, stash]
    NBLK = NB * P
    NEG = 0.2

    nc = bacc.Bacc(num_devices=NCORES, num_swdge_queues=4)

    xT = nc.dram_tensor("xT", [2, P, NPAD], bf16, kind="ExternalInput")
    xTo = nc.dram_tensor("xTo", [2, P, NBLK], bf16, kind="ExternalInput")
    w1e = nc.dram_tensor("w1e", [2, P, W1C], bf16, kind="ExternalInput")
    w1s = nc.dram_tensor("w1s", [2, P, H1], bf16, kind="ExternalInput")
    w2e = nc.dram_tensor("w2e", [2, P, W2C], bf16, kind="ExternalInput")
    negcs = nc.dram_tensor("negcs", [P, W2C], f32, kind="ExternalInput")
    g_idx_d = nc.dram_tensor("g_idx", [P, plan["g_cols"]], i16, kind="ExternalInput")
    dl_idx_d = nc.dram_tensor("dl_idx", [P, plan["d_cols"]], i16, kind="ExternalInput")
    d_fp_d = nc.dram_tensor("d_fp", [NTOT, P], bf16, kind="ExternalInput")
    if has_b1:
        b1_d = nc.dram_tensor("b1r", [P, HID], bf16, kind="ExternalInput")
    if has_b2:
        b2_d = nc.dram_tensor("b2r", [P, OUT], f32, kind="ExternalInput")
    out2 = nc.dram_tensor("out2", [NPC, OUT], f32, kind="ExternalOutput")

    hext1 = nc.dram_tensor("hext1", [NPAD, ROW1], bf16)
    s1dst = nc.dram_tensor("s1dst", [NBLK, ROW2], bf16)
    h2loc = nc.dram_tensor("h2loc", [NBLK, ROW2], bf16)
    h2full = nc.dram_tensor("h2full", [N, ROW2], bf16, addr_space="Shared")

    def sub_ap(t, elem_off, dims_):
        a = t[:]
        return bass.AP(tensor=a.tensor, offset=a.offset + elem_off,
                       ap=[a.ap[0]] + dims_)

    with tile.TileContext(nc, num_cores=NCORES) as tc:
        with tc.tile_pool(name="consts", bufs=1) as cp:
            w1t = []
            w1st = []
            w2t = []
            for k in range(2):
                t1 = cp.tile([P, W1C], bf16, tag=f"w1t{k}")
                nc.sync.dma_start(out=t1[:], in_=w1e[k])
                w1t.append(t1)
                t2 = cp.tile([P, H1], bf16, tag=f"w1st{k}")
                nc.sync.dma_start(out=t2[:], in_=w1s[k])
                w1st.append(t2)
                t3 = cp.tile([P, W2C], bf16, tag=f"w2t{k}")
                nc.sync.dma_start(out=t3[:], in_=w2e[k])
                w2t.append(t3)
            ncs_t = cp.tile([P, W2C], f32)
            nc.sync.dma_start(out=ncs_t[:], in_=negcs[:])
            gidx_t = cp.tile([P, plan["g_cols"]], i16)
            nc.sync.dma_start(out=gidx_t[:], in_=g_idx_d[:])
            dlidx_t = cp.tile([P, plan["d_cols"]], i16)
            nc.sync.dma_start(out=dlidx_t[:], in_=dl_idx_d[:])
            dfp_t = cp.tile([P, NTOT], bf16)
            nc.sync.dma_start(
                out=dfp_t[:],
                in_=bass.AP(tensor=d_fp_d[:].tensor, offset=0,
                            ap=[[1, P], [P, NTOT]]))
            iota_i = cp.tile([P, P], mybir.dt.int32)
            nc.gpsimd.iota(iota_i[:], pattern=[[1, P]], base=0,
                           channel_multiplier=0)
            iota_t = cp.tile([P, P], bf16)
            nc.vector.tensor_copy(out=iota_t[:], in_=iota_i[:])
            pidx_i = cp.tile([P, 1], mybir.dt.int32)
            nc.gpsimd.iota(pidx_i[:], pattern=[[0, 1]], base=0,
                           channel_multiplier=1)
            pidx_f = cp.tile([P, 1], f32)
            nc.vector.tensor_copy(out=pidx_f[:], in_=pidx_i[:])
            ident = cp.tile([P, P], bf16)
            nc.vector.tensor_scalar(out=ident[:], in0=iota_t[:], scalar1=pidx_f[:],
                                    scalar2=None, op0=OP.is_equal)
            b1_t = b2_t = None
            if has_b1:
                b1_t = cp.tile([P, HID], bf16)
                nc.sync.dma_start(out=b1_t[:], in_=b1_d[:])
            if has_b2:
                b2_t = cp.tile([P, OUT], f32)
                nc.sync.dma_start(out=b2_t[:], in_=b2_d[:])

            # ---------------- phase A: h = x @ W1ext (replicated) ----------
            CH = 8
            with (
                tc.tile_pool(name="xc", bufs=4) as xc,
                tc.tile_pool(name="psA", bufs=4, space="PSUM") as psA,
                tc.tile_pool(name="rowp", bufs=6) as rowp,
            ):
                for ch in range(0, NBA, CH):
                    ntc = min(CH, NBA - ch)
                    ck = []
                    for kh in range(2):
                        t_ = xc.tile([P, CH * P], bf16, tag="xchunk")
                        nc.sync.dma_start(out=t_[:, :ntc * P],
                                          in_=xT[kh, :, ch * P:(ch + ntc) * P])
                        ck.append(t_)
                    for j in range(ntc):
                        i = ch + j
                        ps = psA.tile([P, W1C], f32, tag="psA")
                        nc.tensor.matmul(ps[:], ck[0][:, j * P:(j + 1) * P],
                                         w1t[0][:], start=True, stop=False)
                        nc.tensor.matmul(ps[:], ck[1][:, j * P:(j + 1) * P],
                                         w1t[1][:], start=False, stop=True)
                        row = rowp.tile([P, W1C + H1], bf16, tag="row")
                        if i % 2 == 0:
                            nc.scalar.activation(row[:, 0:W1C], ps[:], AF.Copy)
                        else:
                            nc.vector.tensor_copy(out=row[:, 0:W1C], in_=ps[:])
                        # score-lo residual: full-precision score = hi + lo
                        nc.vector.tensor_tensor(
                            out=row[:, W1C:W1C + H1], in0=ps[:, HID:W1C],
                            in1=row[:, HID:W1C], op=OP.subtract)
                        nc.sync.dma_start(
                            out=hext1[i * P:(i + 1) * P, 0:W1C + H1],
                            in_=row[:])
                for ch in range(0, NB, CH):
                    ntc = min(CH, NB - ch)
                    ck = []
                    for kh in range(2):
                        t_ = xc.tile([P, CH * P], bf16, tag="xchunk2")
                        nc.sync.dma_start(out=t_[:, :ntc * P],
                                          in_=xTo[kh, :, ch * P:(ch + ntc) * P])
                        ck.append(t_)
                    for j in range(ntc):
                        i = ch + j
                        ps = psA.tile([P, H1], f32, tag="psA2")
                        nc.tensor.matmul(ps[:], ck[0][:, j * P:(j + 1) * P],
                                         w1st[0][:], start=True, stop=False)
                        nc.tensor.matmul(ps[:], ck[1][:, j * P:(j + 1) * P],
                                         w1st[1][:], start=False, stop=True)
                        row = rowp.tile([P, 2 * H1], bf16, tag="rows")
                        nc.vector.tensor_copy(out=row[:, 0:H1], in_=ps[:])
                        nc.vector.tensor_tensor(
                            out=row[:, H1:2 * H1], in0=ps[:],
                            in1=row[:, 0:H1], op=OP.subtract)
                        nc.sync.dma_start(out=s1dst[i * P:(i + 1) * P, 0:2 * H1],
                                          in_=row[:])

            # ---------------- GAT conv layers ----------------
            def layer(lidx, table, nrows_tab, tdst, H, F, SROW, src_b, src_s,
                      dst_b, dst_s, den_off, epilogue):
                qn = [0]
                C = F // H
                with (
                    tc.tile_pool(name=f"g{lidx}", bufs=3) as gp,
                    tc.tile_pool(name=f"gd{lidx}", bufs=3) as gdp,
                    tc.tile_pool(name=f"wk{lidx}", bufs=2) as wk,
                    tc.tile_pool(name=f"ps{lidx}", bufs=3, space="PSUM") as psp,
                    tc.tile_pool(name=f"pse{lidx}", bufs=2, space="PSUM") as pse,
                    tc.tile_pool(name=f"ep{lidx}", bufs=3) as ep,
                ):
                    def chunks(n):
                        k = (n + GCH - 1) // GCH
                        base, rem = divmod(n, k)
                        out, pos = [], 0
                        for i in range(k):
                            c = base + (1 if i < rem else 0)
                            out.append((pos, c))
                            pos += c
                        return out

                    for g, blocks in enumerate(groups):
                        gbuf = [None, None]
                        for s in range(2):
                            col0, ntg = plan["g_off"][g][s]
                            if ntg == 0:
                                continue
                            gt = gp.tile([P, ntg, SROW], bf16, tag=f"g{s}")
                            base = 0 if s == 0 else SPLIT * SROW
                            inap = bass.AP(
                                tensor=table[:].tensor, offset=base,
                                ap=[[SROW, nrows_tab - (0 if s == 0 else SPLIT)],
                                    [1, SROW]])
                            for c0, cn in chunks(ntg):
                                nc.gpsimd.dma_gather(
                                    gt[:, c0:c0 + cn, :], inap,
                                    gidx_t[:, col0 + c0 * 8:col0 + (c0 + cn) * 8],
                                    cn * P, cn * P, SROW, elem_step=SROW,
                                    queue_num=qn[0] % 4)
                                qn[0] += 1
                            gbuf[s] = gt
                        dcol0, dntg = plan["d_off"][g]
                        gdt = gdp.tile([P, dntg, ROW2], bf16, tag="gd")
                        for c0, cn in chunks(dntg):
                            nc.gpsimd.dma_gather(
                                gdt[:, c0:c0 + cn, :], tdst[:],
                                dlidx_t[:, dcol0 + c0 * 8:dcol0 + (c0 + cn) * 8],
                                cn * P, cn * P, ROW2, elem_step=ROW2,
                                queue_num=qn[0] % 4)
                            qn[0] += 1

                        goff = [0, 0]
                        doff = 0
                        for b in blocks:
                            ntb = int(NT[b, 0] + NT[b, 1])
                            if ntb == 0:
                                continue
                            t0 = int(tile_of[b, 0])
                            # alpha = (shi+dhi) + (slo+dlo), accumulated in f32
                            hl = wk.tile([P, ntb * 2 * H], f32, tag="hl")
                            toff = 0
                            for s in range(2):
                                nts = int(NT[b, s])
                                if nts == 0:
                                    continue
                                gt = gbuf[s]
                                src_ap = sub_ap(gt, goff[s] * SROW + src_b,
                                                [[SROW, nts], [src_s, 2 * H]])
                                dst_ap = sub_ap(gdt, (doff + toff) * ROW2 + dst_b,
                                                [[ROW2, nts], [dst_s, 2 * H]])
                                out_ap = sub_ap(hl, toff * 2 * H,
                                                [[2 * H, nts], [1, 2 * H]])
                                nc.vector.tensor_tensor(out=out_ap, in0=src_ap,
                                                        in1=dst_ap, op=OP.add)
                                toff += nts
                            al = wk.tile([P, ntb * H], f32, tag="al")
                            nc.vector.tensor_tensor(
                                out=sub_ap(al, 0, [[H, ntb], [1, H]]),
                                in0=sub_ap(hl, 0, [[2 * H, ntb], [1, H]]),
                                in1=sub_ap(hl, H, [[2 * H, ntb], [1, H]]),
                                op=OP.add)
                            # leaky relu (fused) then exp
                            nc.vector.scalar_tensor_tensor(
                                out=al[:], in0=al[:], scalar=NEG, in1=al[:],
                                op0=OP.mult, op1=OP.max)
                            wbuf = wk.tile([P, ntb * H], bf16, tag="w")
                            nc.scalar.activation(wbuf[:], al[:], AF.Exp)
                            # one-hot S_T for the whole block in one DVE op
                            st = wk.tile([P, ntb, P], bf16, tag="st")
                            nc.vector.tensor_tensor(
                                out=sub_ap(st, 0, [[P, ntb], [1, P]]),
                                in0=sub_ap(dfp_t, t0, [[1, ntb], [0, P]]),
                                in1=sub_ap(iota_t, 0, [[0, ntb], [1, P]]),
                                op=OP.is_equal)
                            # weight gathered rows; stash w in the den column
                            toff = 0
                            for s in range(2):
                                nts = int(NT[b, s])
                                if nts == 0:
                                    continue
                                gt = gbuf[s]
                                gv = sub_ap(gt, goff[s] * SROW,
                                            [[SROW, nts], [C, H], [1, C]])
                                win = sub_ap(wbuf, toff * H,
                                             [[H, nts], [1, H], [0, C]])
                                nc.vector.tensor_tensor(out=gv, in0=gv, in1=win,
                                                        op=OP.mult)
                                wden = sub_ap(gt, goff[s] * SROW + den_off,
                                              [[SROW, nts], [1, H]])
                                nc.scalar.activation(
                                    wden,
                                    sub_ap(wbuf, toff * H, [[H, nts], [1, H]]),
                                    AF.Copy)
                                toff += nts
                            # fused numerator+denominator matmul per tile
                            ps = psp.tile([P, den_off + H], f32, tag="ps")
                            ti = 0
                            for s in range(2):
                                nts = int(NT[b, s])
                                gt = gbuf[s]
                                for j in range(nts):
                                    gcol = goff[s] + j
                                    nc.tensor.matmul(
                                        ps[:], st[:, ti, :],
                                        gt[:, gcol, 0:den_off + H],
                                        start=(ti == 0), stop=(ti == ntb - 1))
                                    ti += 1
                            rows = min(P, NPC - b * P)
                            epilogue(b, rows, ps, ep, pse)
                            goff[0] += int(NT[b, 0])
                            goff[1] += int(NT[b, 1])
                            doff += ntb

            def epi1(b, rows, ps, ep, pse):
                rden = ep.tile([P, H1], f32, tag="rden")
                nc.vector.reciprocal(rden[:], ps[:, DEN1:DEN1 + H1])
                o = ep.tile([P, HID], bf16, tag="o")
                rb = sub_ap(rden, 0, [[1, H1], [0, C1]])
                num2 = bass.AP(tensor=ps[:].tensor, offset=ps[:].offset,
                               ap=[ps[:].ap[0], [C1, H1], [1, C1]])
                o2d = bass.AP(tensor=o[:].tensor, offset=o[:].offset,
                              ap=[o[:].ap[0], [C1, H1], [1, C1]])
                nc.vector.tensor_tensor(out=o2d, in0=num2, in1=rb, op=OP.mult)
                if b1_t is not None:
                    nc.vector.tensor_tensor(out=o[:], in0=o[:], in1=b1_t[:],
                                            op=OP.add)
                # elu: out = max(o,0) + exp(min(o,0));  the -1 is folded into
                # negcs downstream
                e = ep.tile([P, HID], bf16, tag="e")
                nc.vector.tensor_scalar(out=e[:], in0=o[:], scalar1=0.0,
                                        scalar2=None, op0=OP.min)
                nc.scalar.activation(e[:], e[:], AF.Exp)
                nc.vector.scalar_tensor_tensor(out=o[:], in0=o[:], scalar=0.0,
                                               in1=e[:], op0=OP.max, op1=OP.add)
                h2ps = pse.tile([P, W2C], f32, tag="h2ps")
                for half in range(2):
                    pt = pse.tile([P, P], bf16, tag="pt")
                    nc.tensor.transpose(pt[:], o[:, half * P:(half + 1) * P],
                                        ident[:])
                    et = ep.tile([P, P], bf16, tag="et")
                    if half == 0:
                        nc.scalar.activation(et[:], pt[:], AF.Copy)
                    else:
                        nc.vector.tensor_copy(out=et[:], in_=pt[:])
                    nc.tensor.matmul(h2ps[:], et[:], w2t[half][:],
                                     start=(half == 0), stop=(half == 1))
                h2row = ep.tile([P, ROW2], bf16, tag="h2row")
                # cols: [h2(0:OUT) | shi(OUT) | dhi(OUT+1) | slo | dlo | stash]
                nc.vector.tensor_tensor(out=h2row[:, 0:W2C], in0=h2ps[:],
                                        in1=ncs_t[:], op=OP.add)
                sc32 = ep.tile([P, 2], f32, tag="sc32")
                nc.vector.tensor_tensor(out=sc32[:], in0=h2ps[:, OUT:OUT + 2],
                                        in1=ncs_t[:, OUT:OUT + 2], op=OP.add)
                nc.vector.tensor_tensor(out=h2row[:, W2C:W2C + 2], in0=sc32[:],
                                        in1=h2row[:, OUT:OUT + 2],
                                        op=OP.subtract)
                nc.sync.dma_start(out=h2loc[b * P:b * P + P, :], in_=h2row[:])

            def epi2(b, rows, ps, ep, pse):
                rden = ep.tile([P, 1], f32, tag="rden2")
                nc.vector.reciprocal(rden[:], ps[:, DEN2:DEN2 + 1])
                o = ep.tile([P, OUT], f32, tag="o2")
                if b2_t is not None:
                    nc.vector.scalar_tensor_tensor(
                        out=o[:], in0=ps[:, 0:OUT], scalar=rden[:], in1=b2_t[:],
                        op0=OP.mult, op1=OP.add)
                else:
                    nc.vector.tensor_scalar(out=o[:], in0=ps[:, 0:OUT],
                                            scalar1=rden[:], scalar2=None,
                                            op0=OP.mult)
                nc.sync.dma_start(out=out2[b * P:b * P + rows, :],
                                  in_=o[:rows, :])

            # L1 gathered row: [h(256) | shi(8) | slo(8)]; dst row: [dhi(8)|dlo(8)]
            layer(1, hext1, NPAD, s1dst, H1, HID, ROW1, HID, 1, 0, 1, DEN1, epi1)
            nc.gpsimd.collective_compute(
                "AllGather", mybir.AluOpType.bypass,
                replica_groups=[list(range(NCORES))],
                ins=[h2loc[0:NPC, :]], outs=[h2full[:]],
            )
            # L2 row: [h2(64) | shi | dhi | slo | dlo | stash]: src pair stride 2
            # from col 64, dst pair stride 2 from col 65
            layer(2, h2full, N, h2loc, 1, OUT, ROW2, OUT, 2, OUT + 1, 2, DEN2,
                  epi2)

    nc.finalize()
    return nc


def _host_prep_weights(W1, att1, W2, att2):
    HID = W1.shape[1]
    H1 = att1.shape[1]
    C1 = HID // H1
    OUT = W2.shape[1]
    A_src = np.zeros((HID, H1), np.float32)
    A_dst = np.zeros((HID, H1), np.float32)
    for h in range(H1):
        A_src[h * C1:(h + 1) * C1, h] = att1[0, h, C1:]
        A_dst[h * C1:(h + 1) * C1, h] = att1[0, h, :C1]
    W1ext = np.concatenate([W1, W1 @ A_src], axis=1)
    W1sco = W1 @ A_dst
    a2 = att2[0, 0]
    W2ext = np.concatenate([W2, (W2 @ a2[OUT:])[:, None],
                            (W2 @ a2[:OUT])[:, None]], axis=1)
    return W1ext, W1sco, W2ext


def kernel(x, edge_index, W1, att1, b1, W2, att2, b2):
    import os
    from concourse import mybir
    from concourse.bass_utils import run_bass_kernel_spmd
    ml_bf16 = mybir.dt.np(mybir.dt.bfloat16)

    x = np.asarray(x, np.float32)
    edge_index = np.asarray(edge_index)
    W1 = np.asarray(W1, np.float32)
    att1 = np.asarray(att1, np.float32)
    b1 = np.asarray(b1, np.float32)
    W2 = np.asarray(W2, np.float32)
    att2 = np.asarray(att2, np.float32)
    b2 = np.asarray(b2, np.float32)

    N, IN = x.shape
    HID = W1.shape[1]
    H1 = att1.shape[1]
    C1 = HID // H1
    OUT = W2.shape[1]
    NPC = N // NCORES
    NB = (NPC + P - 1) // P
    NPAD = ((N + P - 1) // P) * P
    NBLK = NB * P

    plan, per_core = _prep(x, edge_index)
    dims = dict(IN=IN, HID=HID, H1=H1, C1=C1, OUT=OUT)
    has_b1 = bool(np.any(b1 != 0))
    has_b2 = bool(np.any(b2 != 0))

    key = (N, IN, HID, H1, OUT, plan["g_cols"], plan["d_cols"], plan["NTOT"],
           has_b1, has_b2, tuple(int(v) for v in plan["NT"].ravel()))
    if key not in _CACHE:
        _CACHE[key] = _build(plan, dims, has_b1, has_b2)
    nc = _CACHE[key]

    W1ext, W1sco, W2ext = _host_prep_weights(W1, att1, W2, att2)
    negcs = np.tile(-W2ext.sum(axis=0, keepdims=True), (P, 1)).astype(np.float32)

    xTfull = np.zeros((IN, NPAD), np.float32)
    xTfull[:, :N] = x.T
    xT = xTfull.reshape(2, P, NPAD).astype(ml_bf16)

    def ktiles(w):
        return np.ascontiguousarray(w.reshape(2, P, -1)).astype(ml_bf16)

    in_maps = []
    for c in range(NCORES):
        xo = np.zeros((IN, NBLK), np.float32)
        xo[:, :NPC] = x[c * NPC:(c + 1) * NPC].T
        m = dict(
            xT=xT,
            xTo=xo.reshape(2, P, NBLK).astype(ml_bf16),
            w1e=ktiles(W1ext),
            w1s=ktiles(W1sco),
            w2e=ktiles(W2ext),
            negcs=negcs,
            g_idx=per_core[c]["g_idx"],
            dl_idx=per_core[c]["dl_idx"],
            d_fp=per_core[c]["d_fp"].astype(ml_bf16),
        )
        if has_b1:
            m["b1r"] = np.tile(b1[None, :], (P, 1)).astype(ml_bf16)
        if has_b2:
            m["b2r"] = np.tile(b2[None, :], (P, 1)).astype(np.float32)
        in_maps.append(m)

    kw = {}
    if os.environ.get("GAT_TRACE"):
        kw = dict(trace=True, tmpdir=os.environ.get("GAT_TRACE_DIR") or None)
    res = run_bass_kernel_spmd(nc, in_maps, list(range(NCORES)), **kw)
    kernel.last_result = res
    out = np.concatenate([res.results[c]["out2"] for c in range(NCORES)], axis=0)
    return np.ascontiguousarray(out.astype(np.float32))


# revision 21
# speedup vs baseline: 1.0339x; 1.0339x over previous
"""2-layer GAT on 8 Trainium2 NeuronCores (Bass/Tile).

Sharding: edges sorted by destination node; nodes partitioned 8 x N/8 across
cores (dst-partitioned edge-parallel). Per-dst softmax groups stay entirely on
one core, so aggregation needs no cross-core reduction. Per 128-node block:
dma_gather of per-edge source feature rows from a replicated table (two halves
so indices fit int16), a one-hot S_T built in one batched DVE op per block,
and the weighted scatter-add + softmax denominator computed as a single
PSUM-accumulated bf16 matmul per 128-edge tile (the per-edge weight is stashed
in a spare column of the gathered rows so numerator and denominator share the
matmul). One AllGather shares the small layer-2 feature table between layers.
"""
import numpy as np

P = 128
NCORES = 8
GRP = 2
GCH = 8  # max gather columns (x128 idxs) per dma_gather call

_CACHE = {}


def _wrap_idx_segments(segs, total_cols):
    arr = np.zeros((16, total_cols), np.int16)
    for off, idx in segs:
        n = len(idx)
        if n:
            arr[:, off:off + n // 16] = idx.reshape(n // 16, 16).T
    return np.tile(arr, (8, 1))


def _prep(x, edge_index):
    N = x.shape[0]
    NPC = N // NCORES
    NB = (NPC + P - 1) // P
    SPLIT = N // 2

    src = np.concatenate([np.asarray(edge_index[0]), np.arange(N, dtype=np.int64)])
    dst = np.concatenate([np.asarray(edge_index[1]), np.arange(N, dtype=np.int64)])
    order = np.argsort(dst, kind="stable")
    s_all = src[order].astype(np.int64)
    d_all = dst[order].astype(np.int64)

    lists = [[[None, None] for _ in range(NB)] for _ in range(NCORES)]
    for c in range(NCORES):
        base = c * NPC
        for b in range(NB):
            e0 = np.searchsorted(d_all, base + b * P)
            e1 = np.searchsorted(d_all, min(base + (b + 1) * P, base + NPC))
            ss, dd = s_all[e0:e1], d_all[e0:e1]
            m = ss < SPLIT
            for s, sel in ((0, m), (1, ~m)):
                sv, dv = ss[sel], dd[sel]
                # ascending dst: the dst-score gather re-reads each row ~deg
                # times back-to-back -> HBM row-buffer hits (src order is
                # effectively random either way)
                o = np.argsort(dv, kind="stable")
                lists[c][b][s] = (sv[o], dv[o])

    NT = np.zeros((NB, 2), np.int64)
    for b in range(NB):
        for s in range(2):
            mx = max(len(lists[c][b][s][0]) for c in range(NCORES))
            NT[b, s] = (mx + P - 1) // P

    groups = [list(range(g, min(g + GRP, NB))) for g in range(0, NB, GRP)]

    tile_of = np.zeros((NB, 2), np.int64)
    t = 0
    for b in range(NB):
        for s in range(2):
            tile_of[b, s] = t
            t += int(NT[b, s])
    NTOT = t

    g_cols, g_off = 0, []
    for g, blocks in enumerate(groups):
        offs = []
        for s in range(2):
            ntg = int(sum(NT[b, s] for b in blocks))
            offs.append((g_cols, ntg))
            g_cols += ntg * 8
        g_off.append(offs)
    d_cols, d_off = 0, []
    for g, blocks in enumerate(groups):
        ntg = int(sum(NT[b, 0] + NT[b, 1] for b in blocks))
        d_off.append((d_cols, ntg))
        d_cols += ntg * 8

    plan = dict(N=N, NPC=NPC, NB=NB, SPLIT=SPLIT, NT=NT, groups=groups,
                tile_of=tile_of, NTOT=NTOT, g_off=g_off, d_off=d_off,
                g_cols=g_cols, d_cols=d_cols)

    per_core = []
    for c in range(NCORES):
        base = c * NPC
        gsegs, dsegs = [], []
        d_fp = np.full((NTOT, P), -1.0, np.float32)
        for g, blocks in enumerate(groups):
            for s in range(2):
                col0, ntg = g_off[g][s]
                idx = np.zeros(ntg * P, np.int64)
                pos = 0
                for b in blocks:
                    ss = lists[c][b][s][0]
                    nslots = int(NT[b, s]) * P
                    idx[pos:pos + len(ss)] = ss - (SPLIT if s == 1 else 0)
                    pos += nslots
                gsegs.append((col0, idx.astype(np.int16)))
            col0, ntg = d_off[g]
            didx = np.zeros(ntg * P, np.int64)
            pos = 0
            for b in blocks:
                for s in range(2):
                    ss, dd = lists[c][b][s]
                    nslots = int(NT[b, s]) * P
                    didx[pos:pos + len(dd)] = dd - base
                    pos += nslots
                    t0 = int(tile_of[b, s])
                    dv = np.full(nslots, -1.0, np.float32)
                    dv[:len(dd)] = (dd - base - b * P).astype(np.float32)
                    d_fp[t0:t0 + int(NT[b, s])] = dv.reshape(int(NT[b, s]), P)
            dsegs.append((col0, didx.astype(np.int16)))
        per_core.append(dict(
            g_idx=_wrap_idx_segments(gsegs, g_cols),
            dl_idx=_wrap_idx_segments(dsegs, d_cols),
            d_fp=d_fp,
        ))
    return plan, per_core


def _build(plan, dims, has_b1, has_b2):
    import concourse.bass as bass
    import concourse.bacc as bacc
    import concourse.tile as tile
    from concourse import mybir

    f32 = mybir.dt.float32
    bf16 = mybir.dt.bfloat16
    i16 = mybir.dt.int16
    AF = mybir.ActivationFunctionType
    OP = mybir.AluOpType

    N, NPC, NB, SPLIT = plan["N"], plan["NPC"], plan["NB"], plan["SPLIT"]
    NT, groups, tile_of, NTOT = (plan["NT"], plan["groups"], plan["tile_of"],
                                 plan["NTOT"])
    HID, H1, C1, OUT = dims["HID"], dims["H1"], dims["C1"], dims["OUT"]
    NPAD = ((N + P - 1) // P) * P
    NBA = NPAD // P
    ROW1 = 384
    W1C = HID + H1          # h | src-score-hi  (lo computed on device)
    DEN1 = HID + 2 * H1     # stash column for the fused denominator
    ROW2 = 128
    W2C = OUT + 2           # h2 | s2src | s2dst  (f32 epilogue values)
    DEN2 = OUT + 4          # h2row: [h2, shi, dhi, slo, dlo, stash]
    NBLK = NB * P
    NEG = 0.2

    nc = bacc.Bacc(num_devices=NCORES, num_swdge_queues=4)

    xT = nc.dram_tensor("xT", [2, P, NPAD], bf16, kind="ExternalInput")
    xTo = nc.dram_tensor("xTo", [2, P, NBLK], bf16, kind="ExternalInput")
    w1e = nc.dram_tensor("w1e", [2, P, W1C], bf16, kind="ExternalInput")
    w1s = nc.dram_tensor("w1s", [2, P, H1], bf16, kind="ExternalInput")
    w2e = nc.dram_tensor("w2e", [2, P, W2C], bf16, kind="ExternalInput")
    negcs = nc.dram_tensor("negcs", [P, W2C], f32, kind="ExternalInput")
    g_idx_d = nc.dram_tensor("g_idx", [P, plan["g_cols"]], i16, kind="ExternalInput")
    dl_idx_d = nc.dram_tensor("dl_idx", [P, plan["d_cols"]], i16, kind="ExternalInput")
    d_fp_d = nc.dram_tensor("d_fp", [NTOT, P], bf16, kind="ExternalInput")
    if has_b1:
        b1_d = nc.dram_tensor("b1r", [P, HID], bf16, kind="ExternalInput")
    if has_b2:
        b2_d = nc.dram_tensor("b2r", [P, OUT], f32, kind="ExternalInput")
    out2 = nc.dram_tensor("out2", [NPC, OUT], f32, kind="ExternalOutput")

    hext1 = nc.dram_tensor("hext1", [NPAD, ROW1], bf16)
    s1dst = nc.dram_tensor("s1dst", [NBLK, ROW2], bf16)
    h2loc = nc.dram_tensor("h2loc", [NBLK, ROW2], bf16)
    h2full = nc.dram_tensor("h2full", [N, ROW2], bf16, addr_space="Shared")

    def sub_ap(t, elem_off, dims_):
        a = t[:]
        return bass.AP(tensor=a.tensor, offset=a.offset + elem_off,
                       ap=[a.ap[0]] + dims_)

    with tile.TileContext(nc, num_cores=NCORES) as tc:
        with tc.tile_pool(name="consts", bufs=1) as cp:
            w1t = []
            w1st = []
            w2t = []
            for k in range(2):
                t1 = cp.tile([P, W1C], bf16, tag=f"w1t{k}")
                nc.sync.dma_start(out=t1[:], in_=w1e[k])
                w1t.append(t1)
                t2 = cp.tile([P, H1], bf16, tag=f"w1st{k}")
                nc.sync.dma_start(out=t2[:], in_=w1s[k])
                w1st.append(t2)
                t3 = cp.tile([P, W2C], bf16, tag=f"w2t{k}")
                nc.sync.dma_start(out=t3[:], in_=w2e[k])
                w2t.append(t3)
            ncs_t = cp.tile([P, W2C], f32)
            nc.sync.dma_start(out=ncs_t[:], in_=negcs[:])
            gidx_t = cp.tile([P, plan["g_cols"]], i16)
            nc.sync.dma_start(out=gidx_t[:], in_=g_idx_d[:])
            dlidx_t = cp.tile([P, plan["d_cols"]], i16)
            nc.sync.dma_start(out=dlidx_t[:], in_=dl_idx_d[:])
            dfp_t = cp.tile([P, NTOT], bf16)
            nc.sync.dma_start(
                out=dfp_t[:],
                in_=bass.AP(tensor=d_fp_d[:].tensor, offset=0,
                            ap=[[1, P], [P, NTOT]]))
            iota_i = cp.tile([P, P], mybir.dt.int32)
            nc.gpsimd.iota(iota_i[:], pattern=[[1, P]], base=0,
                           channel_multiplier=0)
            iota_t = cp.tile([P, P], bf16)
            nc.vector.tensor_copy(out=iota_t[:], in_=iota_i[:])
            pidx_i = cp.tile([P, 1], mybir.dt.int32)
            nc.gpsimd.iota(pidx_i[:], pattern=[[0, 1]], base=0,
                           channel_multiplier=1)
            pidx_f = cp.tile([P, 1], f32)
            nc.vector.tensor_copy(out=pidx_f[:], in_=pidx_i[:])
            ident = cp.tile([P, P], bf16)
            nc.vector.tensor_scalar(out=ident[:], in0=iota_t[:], scalar1=pidx_f[:],
                                    scalar2=None, op0=OP.is_equal)
            b1_t = b2_t = None
            if has_b1:
                b1_t = cp.tile([P, HID], bf16)
                nc.sync.dma_start(out=b1_t[:], in_=b1_d[:])
            if has_b2:
                b2_t = cp.tile([P, OUT], f32)
                nc.sync.dma_start(out=b2_t[:], in_=b2_d[:])

            # ---------------- phase A: h = x @ W1ext (replicated) ----------
            CH = 8
            with (
                tc.tile_pool(name="xc", bufs=4) as xc,
                tc.tile_pool(name="psA", bufs=4, space="PSUM") as psA,
                tc.tile_pool(name="rowp", bufs=6) as rowp,
            ):
                for ch in range(0, NBA, CH):
                    ntc = min(CH, NBA - ch)
                    ck = []
                    for kh in range(2):
                        t_ = xc.tile([P, CH * P], bf16, tag="xchunk")
                        nc.sync.dma_start(out=t_[:, :ntc * P],
                                          in_=xT[kh, :, ch * P:(ch + ntc) * P])
                        ck.append(t_)
                    for j in range(ntc):
                        i = ch + j
                        ps = psA.tile([P, W1C], f32, tag="psA")
                        nc.tensor.matmul(ps[:], ck[0][:, j * P:(j + 1) * P],
                                         w1t[0][:], start=True, stop=False)
                        nc.tensor.matmul(ps[:], ck[1][:, j * P:(j + 1) * P],
                                         w1t[1][:], start=False, stop=True)
                        row = rowp.tile([P, W1C + H1], bf16, tag="row")
                        if i % 2 == 0:
                            nc.scalar.activation(row[:, 0:W1C], ps[:], AF.Copy)
                        else:
                            nc.vector.tensor_copy(out=row[:, 0:W1C], in_=ps[:])
                        # score-lo residual: full-precision score = hi + lo
                        nc.vector.tensor_tensor(
                            out=row[:, W1C:W1C + H1], in0=ps[:, HID:W1C],
                            in1=row[:, HID:W1C], op=OP.subtract)
                        nc.sync.dma_start(
                            out=hext1[i * P:(i + 1) * P, 0:W1C + H1],
                            in_=row[:])
                for ch in range(0, NB, CH):
                    ntc = min(CH, NB - ch)
                    ck = []
                    for kh in range(2):
                        t_ = xc.tile([P, CH * P], bf16, tag="xchunk2")
                        nc.sync.dma_start(out=t_[:, :ntc * P],
                                          in_=xTo[kh, :, ch * P:(ch + ntc) * P])
                        ck.append(t_)
                    for j in range(ntc):
                        i = ch + j
                        ps = psA.tile([P, H1], f32, tag="psA2")
                        nc.tensor.matmul(ps[:], ck[0][:, j * P:(j + 1) * P],
                                         w1st[0][:], start=True, stop=False)
                        nc.tensor.matmul(ps[:], ck[1][:, j * P:(j + 1) * P],
                                         w1st[1][:], start=False, stop=True)
                        row = rowp.tile([P, 2 * H1], bf16, tag="rows")
                        nc.vector.tensor_copy(out=row[:, 0:H1], in_=ps[:])
                        nc.vector.tensor_tensor(
                            out=row[:, H1:2 * H1], in0=ps[:],
                            in1=row[:, 0:H1], op=OP.subtract)
                        nc.sync.dma_start(out=s1dst[i * P:(i + 1) * P, 0:2 * H1],
                                          in_=row[:])

            # ---------------- GAT conv layers ----------------
            def layer(lidx, table, nrows_tab, tdst, H, F, SROW, src_b, src_s,
                      dst_b, dst_s, den_off, epilogue):
                qn = [0]
                C = F // H
                with (
                    tc.tile_pool(name=f"g{lidx}", bufs=3) as gp,
                    tc.tile_pool(name=f"gd{lidx}", bufs=3) as gdp,
                    tc.tile_pool(name=f"wk{lidx}", bufs=2) as wk,
                    tc.tile_pool(name=f"ps{lidx}", bufs=3, space="PSUM") as psp,
                    tc.tile_pool(name=f"pse{lidx}", bufs=2, space="PSUM") as pse,
                    tc.tile_pool(name=f"ep{lidx}", bufs=3) as ep,
                ):
                    def chunks(n):
                        k = (n + GCH - 1) // GCH
                        base, rem = divmod(n, k)
                        out, pos = [], 0
                        for i in range(k):
                            c = base + (1 if i < rem else 0)
                            out.append((pos, c))
                            pos += c
                        return out

                    for g, blocks in enumerate(groups):
                        gbuf = [None, None]
                        for s in range(2):
                            col0, ntg = plan["g_off"][g][s]
                            if ntg == 0:
                                continue
                            gt = gp.tile([P, ntg, SROW], bf16, tag=f"g{s}")
                            base = 0 if s == 0 else SPLIT * SROW
                            inap = bass.AP(
                                tensor=table[:].tensor, offset=base,
                                ap=[[SROW, nrows_tab - (0 if s == 0 else SPLIT)],
                                    [1, SROW]])
                            for c0, cn in chunks(ntg):
                                nc.gpsimd.dma_gather(
                                    gt[:, c0:c0 + cn, :], inap,
                                    gidx_t[:, col0 + c0 * 8:col0 + (c0 + cn) * 8],
                                    cn * P, cn * P, SROW, elem_step=SROW,
                                    queue_num=qn[0] % 4)
                                qn[0] += 1
                            gbuf[s] = gt
                        dcol0, dntg = plan["d_off"][g]
                        gdt = gdp.tile([P, dntg, ROW2], bf16, tag="gd")
                        for c0, cn in chunks(dntg):
                            nc.gpsimd.dma_gather(
                                gdt[:, c0:c0 + cn, :], tdst[:],
                                dlidx_t[:, dcol0 + c0 * 8:dcol0 + (c0 + cn) * 8],
                                cn * P, cn * P, ROW2, elem_step=ROW2,
                                queue_num=qn[0] % 4)
                            qn[0] += 1

                        goff = [0, 0]
                        doff = 0
                        for b in blocks:
                            ntb = int(NT[b, 0] + NT[b, 1])
                            if ntb == 0:
                                continue
                            t0 = int(tile_of[b, 0])
                            # alpha = (shi+dhi) + (slo+dlo), accumulated in f32
                            hl = wk.tile([P, ntb * 2 * H], f32, tag="hl")
                            toff = 0
                            for s in range(2):
                                nts = int(NT[b, s])
                                if nts == 0:
                                    continue
                                gt = gbuf[s]
                                src_ap = sub_ap(gt, goff[s] * SROW + src_b,
                                                [[SROW, nts], [src_s, 2 * H]])
                                dst_ap = sub_ap(gdt, (doff + toff) * ROW2 + dst_b,
                                                [[ROW2, nts], [dst_s, 2 * H]])
                                out_ap = sub_ap(hl, toff * 2 * H,
                                                [[2 * H, nts], [1, 2 * H]])
                                nc.vector.tensor_tensor(out=out_ap, in0=src_ap,
                                                        in1=dst_ap, op=OP.add)
                                toff += nts
                            al = wk.tile([P, ntb * H], f32, tag="al")
                            nc.vector.tensor_tensor(
                                out=sub_ap(al, 0, [[H, ntb], [1, H]]),
                                in0=sub_ap(hl, 0, [[2 * H, ntb], [1, H]]),
                                in1=sub_ap(hl, H, [[2 * H, ntb], [1, H]]),
                                op=OP.add)
                            # leaky relu (fused) then exp
                            nc.vector.scalar_tensor_tensor(
                                out=al[:], in0=al[:], scalar=NEG, in1=al[:],
                                op0=OP.mult, op1=OP.max)
                            wbuf = wk.tile([P, ntb * H], bf16, tag="w")
                            nc.scalar.activation(wbuf[:], al[:], AF.Exp)
                            # one-hot S_T for the whole block in one DVE op
                            st = wk.tile([P, ntb, P], bf16, tag="st")
                            nc.vector.tensor_tensor(
                                out=sub_ap(st, 0, [[P, ntb], [1, P]]),
                                in0=sub_ap(dfp_t, t0, [[1, ntb], [0, P]]),
                                in1=sub_ap(iota_t, 0, [[0, ntb], [1, P]]),
                                op=OP.is_equal)
                            # weight gathered rows; stash w in the den column
                            toff = 0
                            for s in range(2):
                                nts = int(NT[b, s])
                                if nts == 0:
                                    continue
                                gt = gbuf[s]
                                gv = sub_ap(gt, goff[s] * SROW,
                                            [[SROW, nts], [C, H], [1, C]])
                                win = sub_ap(wbuf, toff * H,
                                             [[H, nts], [1, H], [0, C]])
                                nc.vector.tensor_tensor(out=gv, in0=gv, in1=win,
                                                        op=OP.mult)
                                wden = sub_ap(gt, goff[s] * SROW + den_off,
                                              [[SROW, nts], [1, H]])
                                nc.scalar.activation(
                                    wden,
                                    sub_ap(wbuf, toff * H, [[H, nts], [1, H]]),
                                    AF.Copy)
                                toff += nts
                            # fused numerator+denominator matmul per tile
                            ps = psp.tile([P, den_off + H], f32, tag="ps")
                            ti = 0
                            for s in range(2):
                                nts = int(NT[b, s])
                                gt = gbuf[s]
                                for j in range(nts):
                                    gcol = goff[s] + j
                                    nc.tensor.matmul(
                                        ps[:], st[:, ti, :],
                                        gt[:, gcol, 0:den_off + H],
                                        start=(ti == 0), stop=(ti == ntb - 1))
                                    ti += 1
                            rows = min(P, NPC - b * P)
                            epilogue(b, rows, ps, ep, pse)
                            goff[0] += int(NT[b, 0])
                            goff[1] += int(NT[b, 1])
                            doff += ntb

            def epi1(b, rows, ps, ep, pse):
                rden = ep.tile([P, H1], f32, tag="rden")
                nc.vector.reciprocal(rden[:], ps[:, DEN1:DEN1 + H1])
                o = ep.tile([P, HID], bf16, tag="o")
                rb = sub_ap(rden, 0, [[1, H1], [0, C1]])
                num2 = bass.AP(tensor=ps[:].tensor, offset=ps[:].offset,
                               ap=[ps[:].ap[0], [C1, H1], [1, C1]])
                o2d = bass.AP(tensor=o[:].tensor, offset=o[:].offset,
                              ap=[o[:].ap[0], [C1, H1], [1, C1]])
                nc.vector.tensor_tensor(out=o2d, in0=num2, in1=rb, op=OP.mult)
                if b1_t is not None:
                    nc.vector.tensor_tensor(out=o[:], in0=o[:], in1=b1_t[:],
                                            op=OP.add)
                # elu: out = max(o,0) + exp(min(o,0));  the -1 is folded into
                # negcs downstream
                e = ep.tile([P, HID], bf16, tag="e")
                nc.vector.tensor_scalar(out=e[:], in0=o[:], scalar1=0.0,
                                        scalar2=None, op0=OP.min)
                nc.scalar.activation(e[:], e[:], AF.Exp)
                nc.vector.scalar_tensor_tensor(out=o[:], in0=o[:], scalar=0.0,
                                               in1=e[:], op0=OP.max, op1=OP.add)
                h2ps = pse.tile([P, W2C], f32, tag="h2ps")
                for half in range(2):
                    pt = pse.tile([P, P], bf16, tag="pt")
                    nc.tensor.transpose(pt[:], o[:, half * P:(half + 1) * P],
                                        ident[:])
                    et = ep.tile([P, P], bf16, tag="et")
                    if half == 0:
                        nc.scalar.activation(et[:], pt[:], AF.Copy)
                    else:
                        nc.vector.tensor_copy(out=et[:], in_=pt[:])
                    nc.tensor.matmul(h2ps[:], et[:], w2t[half][:],
                                     start=(half == 0), stop=(half == 1))
                h2row = ep.tile([P, ROW2], bf16, tag="h2row")
                # cols: [h2(0:OUT) | shi(OUT) | dhi(OUT+1) | slo | dlo | stash]
                nc.vector.tensor_tensor(out=h2row[:, 0:W2C], in0=h2ps[:],
                                        in1=ncs_t[:], op=OP.add)
                sc32 = ep.tile([P, 2], f32, tag="sc32")
                nc.vector.tensor_tensor(out=sc32[:], in0=h2ps[:, OUT:OUT + 2],
                                        in1=ncs_t[:, OUT:OUT + 2], op=OP.add)
                nc.vector.tensor_tensor(out=h2row[:, W2C:W2C + 2], in0=sc32[:],
                                        in1=h2row[:, OUT:OUT + 2],
                                        op=OP.subtract)
                nc.sync.dma_start(out=h2loc[b * P:b * P + P, :], in_=h2row[:])

            def epi2(b, rows, ps, ep, pse):
                rden = ep.tile([P, 1], f32, tag="rden2")
                nc.vector.reciprocal(rden[:], ps[:, DEN2:DEN2 + 1])
                o = ep.tile([P, OUT], f32, tag="o2")
                if b2_t is not None:
                    nc.vector.scalar_tensor_tensor(
                        out=o[:], in0=ps[:, 0:OUT], scalar=rden[:], in1=b2_t[:],
                        op0=OP.mult, op1=OP.add)
                else:
                    nc.vector.tensor_scalar(out=o[:], in0=ps[:, 0:OUT],
                                            scalar1=rden[:], scalar2=None,
                                            op0=OP.mult)
                nc.sync.dma_start(out=out2[b * P:b * P + rows, :],
                                  in_=o[:rows, :])

            # L1 gathered row: [h(256) | shi(8) | slo(8)]; dst row: [dhi(8)|dlo(8)]
            layer(1, hext1, NPAD, s1dst, H1, HID, ROW1, HID, 1, 0, 1, DEN1, epi1)
            nc.gpsimd.collective_compute(
                "AllGather", mybir.AluOpType.bypass,
                replica_groups=[list(range(NCORES))],
                ins=[h2loc[0:NPC, :]], outs=[h2full[:]],
            )
            # L2 row: [h2(64) | shi | dhi | slo | dlo | stash]: src pair stride 2
            # from col 64, dst pair stride 2 from col 65
            layer(2, h2full, N, h2loc, 1, OUT, ROW2, OUT, 2, OUT + 1, 2, DEN2,
                  epi2)

    nc.finalize()
    return nc


def _host_prep_weights(W1, att1, W2, att2):
    HID = W1.shape[1]
    H1 = att1.shape[1]
    C1 = HID // H1
    OUT = W2.shape[1]
    A_src = np.zeros((HID, H1), np.float32)
    A_dst = np.zeros((HID, H1), np.float32)
    for h in range(H1):
        A_src[h * C1:(h + 1) * C1, h] = att1[0, h, C1:]
        A_dst[h * C1:(h + 1) * C1, h] = att1[0, h, :C1]
    W1ext = np.concatenate([W1, W1 @ A_src], axis=1)
    W1sco = W1 @ A_dst
    a2 = att2[0, 0]
    W2ext = np.concatenate([W2, (W2 @ a2[OUT:])[:, None],
                            (W2 @ a2[:OUT])[:, None]], axis=1)
    return W1ext, W1sco, W2ext


def kernel(x, edge_index, W1, att1, b1, W2, att2, b2):
    import os
    from concourse import mybir
    from concourse.bass_utils import run_bass_kernel_spmd
    ml_bf16 = mybir.dt.np(mybir.dt.bfloat16)

    x = np.asarray(x, np.float32)
    edge_index = np.asarray(edge_index)
    W1 = np.asarray(W1, np.float32)
    att1 = np.asarray(att1, np.float32)
    b1 = np.asarray(b1, np.float32)
    W2 = np.asarray(W2, np.float32)
    att2 = np.asarray(att2, np.float32)
    b2 = np.asarray(b2, np.float32)

    N, IN = x.shape
    HID = W1.shape[1]
    H1 = att1.shape[1]
    C1 = HID // H1
    OUT = W2.shape[1]
    NPC = N // NCORES
    NB = (NPC + P - 1) // P
    NPAD = ((N + P - 1) // P) * P
    NBLK = NB * P

    plan, per_core = _prep(x, edge_index)
    dims = dict(IN=IN, HID=HID, H1=H1, C1=C1, OUT=OUT)
    has_b1 = bool(np.any(b1 != 0))
    has_b2 = bool(np.any(b2 != 0))

    key = (N, IN, HID, H1, OUT, plan["g_cols"], plan["d_cols"], plan["NTOT"],
           has_b1, has_b2, tuple(int(v) for v in plan["NT"].ravel()))
    if key not in _CACHE:
        _CACHE[key] = _build(plan, dims, has_b1, has_b2)
    nc = _CACHE[key]

    W1ext, W1sco, W2ext = _host_prep_weights(W1, att1, W2, att2)
    negcs = np.tile(-W2ext.sum(axis=0, keepdims=True), (P, 1)).astype(np.float32)

    xTfull = np.zeros((IN, NPAD), np.float32)
    xTfull[:, :N] = x.T
    xT = xTfull.reshape(2, P, NPAD).astype(ml_bf16)

    def ktiles(w):
        return np.ascontiguousarray(w.reshape(2, P, -1)).astype(ml_bf16)

    in_maps = []
    for c in range(NCORES):
        xo = np.zeros((IN, NBLK), np.float32)
        xo[:, :NPC] = x[c * NPC:(c + 1) * NPC].T
        m = dict(
            xT=xT,
            xTo=xo.reshape(2, P, NBLK).astype(ml_bf16),
            w1e=ktiles(W1ext),
            w1s=ktiles(W1sco),
            w2e=ktiles(W2ext),
            negcs=negcs,
            g_idx=per_core[c]["g_idx"],
            dl_idx=per_core[c]["dl_idx"],
            d_fp=per_core[c]["d_fp"].astype(ml_bf16),
        )
        if has_b1:
            m["b1r"] = np.tile(b1[None, :], (P, 1)).astype(ml_bf16)
        if has_b2:
            m["b2r"] = np.tile(b2[None, :], (P, 1)).astype(np.float32)
        in_maps.append(m)

    kw = {}
    if os.environ.get("GAT_TRACE"):
        kw = dict(trace=True, tmpdir=os.environ.get("GAT_TRACE_DIR") or None)
    res = run_bass_kernel_spmd(nc, in_maps, list(range(NCORES)), **kw)
    kernel.last_result = res
    out = np.concatenate([res.results[c]["out2"] for c in range(NCORES)], axis=0)
    return np.ascontiguousarray(out.astype(np.float32))
